# revision 1
# baseline (speedup 1.0000x reference)
"""Trainium2 Bass kernel for the 2-layer GAT node-classification head.

The reference reads only h2[mask_idx] and x[mask_idx] for the classifier, so
the exact computation collapses to mask_idx's 2-hop in-neighborhood:

  layer 1: h1 = x @ W1 is needed only at sources of in-edges of V1
           (V1 = sources of mask's in-edges), one row per edge in S2.
  layer 2: h2 = elu(gat1) @ W2 is needed only at rows V1, and the final
           classifier (fc -> cls, two consecutive affine maps) folds into a
           single [1536, 2] matrix on the host, so layer-2's GEMM contracts
           into W2 @ fold (4 columns: 2 logits + a_src2 + a_dst2).

Sharding over 8 cores:
  - layer-1 GEMM + attention by head (H1=8 -> head i on core i)
  - layer-2 folded GEMM by contraction block (core i contracts the head-i
    block of elu(h1)); one AllReduce(add) of the small partial
  - everything after the AllReduce is tiny and runs redundantly on all cores

Host preprocessing: gather + transpose the needed x rows (index-select is
part of sharding), fold attention vectors and the classifier into the weight
matrices, and build one-hot scatter matrices plus a uniform-stride edge
layout so segment softmax lowers to batched strided reductions.
"""

import numpy as np

import concourse.bass as bass
import concourse.mybir as mybir
import concourse.tile as tile
from concourse import bacc
from concourse.bass_utils import run_bass_kernel_spmd
from concourse.masks import make_identity

NCORES = 8
P = 128
C = 768          # input feature dim
H1 = 8           # layer-1 heads
OUT = 768        # per-head feature dim
KC = C // P      # 6 k-chunks of 128 over a 768 contraction
W2F = 4          # folded layer-2 rhs cols: [cls0 cls1 a_src2 a_dst2]
NEG = -1.0e30    # padding logit

f32 = mybir.dt.float32
f32r = mybir.dt.float32r
bf16 = mybir.dt.bfloat16
i32 = mybir.dt.int32
GEMM_DT = f32r   # single-pass fp32 matmul: full DMA bytes, 4x PE rate


# ---------------------------------------------------------------- host graph
def _preprocess(edge_index, mask_idx, n_nodes):
    """Extract the 2-hop in-neighborhood of mask_idx and pack it into
    uniform-stride group tiles. Everything in meta is compile-time python."""
    ei = np.asarray(edge_index).astype(np.int64)
    m = int(np.asarray(mask_idx))
    src_all = np.concatenate([ei[0], np.arange(n_nodes, dtype=np.int64)])
    dst_all = np.concatenate([ei[1], np.arange(n_nodes, dtype=np.int64)])

    s1_pos = np.nonzero(dst_all == m)[0]          # in-edges of m (incl self-loop)
    s1_src = src_all[s1_pos].tolist()
    v1 = list(dict.fromkeys(s1_src))              # unique sources, first-occurrence
    v1n = len(v1)
    v1p = max(v1n, 2)
    assert v1n <= P, f"in-degree of mask node too large: {v1n}"
    v1_row = {v: r for r, v in enumerate(v1)}
    s1n = len(s1_src)
    n_s1t = max(1, -(-s1n // P))
    s1p = n_s1t * P
    assert s1p <= 512, f"mask in-degree {s1n} exceeds 512"
    # layer-2 gather is the identity when every in-edge has a distinct source
    s1_ident = s1n == v1n

    # S2: in-edges of each v in V1, at uniform stride gmax within tiles
    groups = [src_all[np.nonzero(dst_all == v)[0]].tolist() for v in v1]
    gmax = max(len(g) for g in groups)
    assert gmax <= P, f"in-degree {gmax} exceeds {P}"
    gpt = P // gmax                               # groups per 128-slot tile
    n_et = -(-v1n // gpt)
    s2p = n_et * P

    src_ids = np.zeros(s2p, np.int64)             # padded with node 0
    m01 = np.zeros((s2p, v1p), np.float32)
    padbias = np.full((H1, s2p), NEG, np.float32)
    ngs = []                                      # groups in each tile
    for t in range(n_et):
        gs = groups[t * gpt:(t + 1) * gpt]
        ngs.append(len(gs))
        for j, srcs in enumerate(gs):
            v_row = t * gpt + j
            lo = t * P + j * gmax
            src_ids[lo:lo + len(srcs)] = srcs
            m01[lo:lo + len(srcs), v_row] = 1.0
            padbias[:, lo:lo + len(srcs)] = 0.0

    v1_ids = np.zeros(v1p, np.int64)
    v1_ids[:v1n] = np.array(v1, np.int64)

    g_mat = np.zeros((v1p, s1p), np.float32)      # a_src2 gather (src of S1 edge)
    gm_mat = np.zeros((v1p, s1p), np.float32)     # a_dst2 broadcast (row of m)
    for e, s in enumerate(s1_src):
        g_mat[v1_row[s], e] = 1.0
        gm_mat[v1_row[m], e] = 1.0
    gt_mat = np.ascontiguousarray(g_mat.T)        # [s1p, v1p]

    meta = dict(m=m, v1n=v1n, v1p=v1p, s1n=s1n, s1p=s1p, n_s1t=n_s1t,
                n_et=n_et, gmax=gmax, ngs=tuple(ngs), s1_ident=s1_ident)
    host = dict(src_ids=src_ids, v1_ids=v1_ids, m01=m01, padbias=padbias,
                m01t=np.ascontiguousarray(m01.T), g=g_mat, gm=gm_mat,
                gt=gt_mat)
    return meta, host


def _chunked(w):
    """[K, N] -> [128, (K//128)*N] with chunk-major free layout for one DMA."""
    k, n = w.shape
    assert k % P == 0
    return np.ascontiguousarray(
        w.reshape(k // P, P, n).transpose(1, 0, 2).reshape(P, (k // P) * n))


def _colmajor(v):
    """[768] -> [128, 6] column-chunk layout."""
    return np.ascontiguousarray(v.reshape(KC, P).T)


def _const_layout(meta):
    """Column layout of the packed-constants tensor, shared host/build."""
    v1p, s1p, n_s1t = meta["v1p"], meta["s1p"], meta["n_s1t"]
    s2p = meta["n_et"] * P
    pieces = [
        ("xvt", P, KC * v1p),
        ("ones", NCORES, 1),
        ("wd1", P, KC * H1),
        ("w2f", P, KC * W2F),
        ("wfb", P, KC * 2),
        ("b1", P, KC),
        ("xm", P, KC),
        ("m01", P, meta["n_et"] * v1p),
        ("m01t", v1p, s2p),
        ("g", v1p, s1p),
        ("gm", v1p, s1p),
        ("gt", P, n_s1t * v1p),
        ("padbias", H1, s2p),
        ("bias3", 1, 2),
        ("head", H1, 1),
    ]
    lay, off = {}, 0
    for name, rows, cols in pieces:
        lay[name] = (rows, off, cols)
        off += cols
    return lay, off


# ---------------------------------------------------------------- bass build
def _build(meta):
    v1p, s1p, n_s1t, n_et = meta["v1p"], meta["s1p"], meta["n_s1t"], meta["n_et"]
    gmax, ngs = meta["gmax"], meta["ngs"]
    s2p = n_et * P
    packed = n_s1t == 1
    ccw = P * 3 if packed else 2 * v1p + s1p      # AllGather payload
    lay, cw = _const_layout(meta)

    nc = bacc.Bacc("TRN2", target_bir_lowering=False, debug=False,
                   enable_asserts=True, num_devices=NCORES)

    d_xga = nc.dram_tensor("xga", [P, KC * (s2p + H1)], GEMM_DT,
                           kind="ExternalInput")
    d_cst = nc.dram_tensor("cst", [P, cw], f32, kind="ExternalInput")
    d_w1 = [nc.dram_tensor(f"w1c{c}", [P, OUT], GEMM_DT, kind="ExternalInput")
            for c in range(KC)]
    d_res = nc.dram_tensor("res", [1, 2], f32, kind="ExternalOutput")

    with tile.TileContext(nc) as tc:
        with (
            tc.tile_pool(name="const", bufs=1) as cpool,
            tc.tile_pool(name="sbuf", bufs=2) as sb,
            tc.tile_pool(name="big", bufs=1) as bigp,
            tc.tile_pool(name="ps", bufs=1, space="PSUM") as ps,
            tc.tile_pool(name="dram", bufs=1, space="DRAM") as dr,
        ):
            # ---- input loads: critical pieces first ----
            xga_sb = bigp.tile([P, KC, s2p + H1], GEMM_DT, tag="xga")
            nc.sync.dma_start(out=xga_sb[:], in_=d_xga[:].rearrange(
                "p (k n) -> p k n", k=KC))
            cst = cpool.tile([P, cw], f32, tag="cst")
            nc.sync.dma_start(out=cst[:], in_=d_cst[:])
            w1_sb = [bigp.tile([P, OUT], GEMM_DT, tag=f"w1_{c}", name=f"w1_{c}")
                     for c in range(KC)]
            for c in range(KC):
                nc.sync.dma_start(out=w1_sb[c][:], in_=d_w1[c][:])

            def cv(name):
                rows, off, cols = lay[name]
                return cst[0:rows, off:off + cols]

            xvt_v = cv("xvt").rearrange("p (k n) -> p k n", k=KC)
            ones_v = cv("ones")
            wd1_v = cv("wd1").rearrange("p (k n) -> p k n", k=KC)
            w2f_v = cv("w2f").rearrange("p (k n) -> p k n", k=KC)
            wfb_v = cv("wfb").rearrange("p (k n) -> p k n", k=KC)
            b1_v = cv("b1")
            xm_v = cv("xm")
            m01_v = cv("m01").rearrange("p (t n) -> p t n", t=n_et)
            m01t_v = cv("m01t")
            g_v = cv("g")
            gm_v = cv("gm")
            gt_v = cv("gt").rearrange("p (k n) -> p k n", k=n_s1t)
            pad_v = cv("padbias")
            bias3_v = cv("bias3")
            head_v = cv("head")

            ident = cpool.tile([P, P], f32, tag="ident")
            make_identity(nc, ident[:])

            # ---- attention inputs: a_src per edge, a_dst per node ----
            asT_sb = []
            for t in range(n_et):
                ap_s = ps.tile([P, H1], f32, tag="mm_b", name="ap_s")
                for c in range(KC):
                    nc.tensor.matmul(out=ap_s[:],
                                     lhsT=xga_sb[:, c, t * P:(t + 1) * P],
                                     rhs=xga_sb[:, c, s2p:s2p + H1],
                                     start=(c == 0), stop=(c == KC - 1))
                asb = sb.tile([P, H1], f32, tag=f"as_{t}", name=f"as_{t}")
                nc.vector.tensor_copy(out=asb[:], in_=ap_s[:])
                at = ps.tile([H1, P], f32, tag="tp", bufs=2, name="at")
                nc.tensor.transpose(out=at[:], in_=asb[:], identity=ident[:])
                at2 = sb.tile([H1, P], f32, tag=f"asT_{t}", name=f"asT_{t}")
                nc.vector.tensor_copy(out=at2[:], in_=at[:])
                asT_sb.append(at2)
            adv_ps = ps.tile([v1p, H1], f32, tag="mm_b", name="adv")
            for c in range(KC):
                nc.tensor.matmul(out=adv_ps[:], lhsT=xvt_v[:, c, :],
                                 rhs=wd1_v[:, c, :],
                                 start=(c == 0), stop=(c == KC - 1))
            adv_sb = sb.tile([v1p, H1], f32, tag="adv_sb")
            nc.vector.tensor_copy(out=adv_sb[:], in_=adv_ps[:])

            # ---- layer-1 logits + batched segment softmax (all heads) ----
            logit = sb.tile([H1, s2p], f32, tag="logit")
            for t in range(n_et):
                adT = ps.tile([H1, P], f32, tag="tp", bufs=2, name="adT")
                nc.tensor.matmul(out=adT[:], lhsT=adv_sb[:],
                                 rhs=m01t_v[:, t * P:(t + 1) * P],
                                 start=True, stop=True)
                nc.vector.tensor_add(out=logit[:, t * P:(t + 1) * P],
                                     in0=asT_sb[t][:], in1=adT[:])
            # leaky relu + padding mask
            tmp = sb.tile([H1, s2p], f32, tag="ltmp")
            nc.vector.tensor_scalar_mul(out=tmp[:], in0=logit[:], scalar1=0.2)
            nc.vector.tensor_tensor(out=logit[:], in0=logit[:], in1=tmp[:],
                                    op=mybir.AluOpType.max)
            nc.vector.tensor_add(out=logit[:], in0=logit[:], in1=pad_v)
            # per-group max-shift, exp, normalize (strided batched form)
            for t in range(n_et):
                ng = ngs[t]
                view = logit[:, t * P:t * P + ng * gmax].rearrange(
                    "h (g e) -> h g e", e=gmax)
                mx = sb.tile([H1, ng], f32, tag=f"mx{t}", name=f"mx{t}")
                nc.vector.reduce_max(out=mx[:], in_=view,
                                     axis=mybir.AxisListType.X)
                mxb = mx[:].rearrange("h (g o) -> h g o", o=1).to_broadcast(
                    [H1, ng, gmax])
                nc.vector.tensor_tensor(out=view, in0=view, in1=mxb,
                                        op=mybir.AluOpType.subtract)
            nc.scalar.activation(out=logit[:], in_=logit[:],
                                 func=mybir.ActivationFunctionType.Exp)
            for t in range(n_et):
                ng = ngs[t]
                view = logit[:, t * P:t * P + ng * gmax].rearrange(
                    "h (g e) -> h g e", e=gmax)
                sm = sb.tile([H1, ng], f32, tag=f"sm{t}", name=f"sm{t}")
                nc.vector.reduce_sum(out=sm[:], in_=view,
                                     axis=mybir.AxisListType.X)
                rc = sb.tile([H1, ng], f32, tag=f"rc{t}", name=f"rc{t}")
                nc.vector.reciprocal(out=rc[:], in_=sm[:])
                rcb = rc[:].rearrange("h (g o) -> h g o", o=1).to_broadcast(
                    [H1, ng, gmax])
                nc.vector.tensor_tensor(out=view, in0=view, in1=rcb,
                                        op=mybir.AluOpType.mult)
            # alpha column for this core's head + alpha-scaled selection
            a_sel = []
            for t in range(n_et):
                acol = ps.tile([P, 1], f32, tag="tp", bufs=2, name="acol")
                nc.tensor.matmul(out=acol[:],
                                 lhsT=logit[:, t * P:(t + 1) * P],
                                 rhs=head_v, start=True, stop=True)
                acs = sb.tile([P, 1], f32, tag=f"acol_sb{t}", name=f"acol_sb{t}")
                nc.vector.tensor_copy(out=acs[:], in_=acol[:])
                asel = sb.tile([P, v1p], f32, tag=f"a_sel{t}", name=f"a_sel{t}")
                nc.vector.tensor_scalar(out=asel[:], in0=m01_v[:, t, :],
                                        scalar1=acs[:], scalar2=None,
                                        op0=mybir.AluOpType.mult)
                a_sel.append(asel)

            # ---- the big per-head GEMM1: h1 = x_src @ W1_head ----
            h1_sb = []
            for t in range(n_et):
                hp_a = ps.tile([P, 512], f32, tag="mm_a", name="hp_a")
                hp_b = ps.tile([P, 256], f32, tag="mm_b", name="hp_b")
                for c in range(KC):
                    nc.tensor.matmul(out=hp_a[:],
                                     lhsT=xga_sb[:, c, t * P:(t + 1) * P],
                                     rhs=w1_sb[c][:, 0:512],
                                     start=(c == 0), stop=(c == KC - 1))
                for c in range(KC):
                    nc.tensor.matmul(out=hp_b[:],
                                     lhsT=xga_sb[:, c, t * P:(t + 1) * P],
                                     rhs=w1_sb[c][:, 512:OUT],
                                     start=(c == 0), stop=(c == KC - 1))
                h1t = sb.tile([P, OUT], f32, tag=f"h1_{t}", name=f"h1_{t}")
                nc.vector.tensor_copy(out=h1t[:, 0:512], in_=hp_a[:])
                nc.vector.tensor_copy(out=h1t[:, 512:OUT], in_=hp_b[:])
                h1_sb.append(h1t)

            # ---- xm @ Wf_bot partial (independent of the collective) ----
            oxm_ps = ps.tile([1, 2], f32, tag="oxm", name="oxm_ps")
            for c in range(KC):
                nc.tensor.matmul(out=oxm_ps[:], lhsT=xm_v[:, c:c + 1],
                                 rhs=wfb_v[:, c, :],
                                 start=(c == 0), stop=(c == KC - 1))
            oxm_sb = sb.tile([1, 2], f32, tag="oxm_sb")
            nc.vector.tensor_add(out=oxm_sb[:], in0=oxm_ps[:], in1=bias3_v)

            # ---- aggregation + bias, batched elu, folded layer-2 partial ----
            helu = sb.tile([P, KC, v1p], f32, tag="helu")
            assert KC * v1p <= 512
            agg = ps.tile([P, KC * v1p], f32, tag="agg", bufs=2, name="agg")
            for c in range(KC):
                for t in range(n_et):
                    nc.tensor.matmul(out=agg[:, c * v1p:(c + 1) * v1p],
                                     lhsT=h1_sb[t][:, c * P:(c + 1) * P],
                                     rhs=a_sel[t][:], start=(t == 0),
                                     stop=(t == n_et - 1))
            b1b = b1_v.rearrange("p (k o) -> p k o", o=1).to_broadcast(
                [P, KC, v1p])
            nc.vector.tensor_tensor(
                out=helu[:], in0=agg[:].rearrange("p (k n) -> p k n", k=KC),
                in1=b1b, op=mybir.AluOpType.add)
            # elu(x) = max(x,0) + exp(min(x,0)) - 1, one pass over all chunks
            hall = helu[:].rearrange("p k n -> p (k n)")
            mn = sb.tile([P, KC * v1p], f32, tag="mn")
            nc.vector.tensor_scalar_min(out=mn[:], in0=hall, scalar1=0.0)
            nc.scalar.activation(out=mn[:], in_=mn[:],
                                 func=mybir.ActivationFunctionType.Exp)
            nc.vector.tensor_scalar_max(out=hall, in0=hall, scalar1=0.0)
            nc.vector.tensor_add(out=hall, in0=hall, in1=mn[:])
            nc.vector.tensor_scalar_add(out=hall, in0=hall, scalar1=-1.0)
            h2f_ps = ps.tile([v1p, W2F], f32, tag="h2f", name="h2f")
            for c in range(KC):
                nc.tensor.matmul(out=h2f_ps[:], lhsT=helu[:, c, :],
                                 rhs=w2f_v[:, c, :],
                                 start=(c == 0), stop=(c == KC - 1))
            h2f_part = sb.tile([v1p, W2F], f32, tag="h2f_part")
            nc.vector.tensor_copy(out=h2f_part[:], in_=h2f_ps[:])

            # layer-2 logits are linear in h2f -> fold into the AllGather
            cc_in = dr.tile([1, ccw], f32, tag="cc_in", name="cc_in")
            cc_out = dr.tile([1, NCORES * ccw], f32, tag="cc_out",
                             name="cc_out")
            if packed:
                # transposed logits land partition-major next to h2f cols so
                # one staging tile covers the whole payload in a single DMA
                lgT_ps = ps.tile([P, 1], f32, tag="tp", bufs=2, name="lgT")
                nc.tensor.matmul(out=lgT_ps[:], lhsT=g_v[:, 0:P],
                                 rhs=h2f_part[:, 2:3], start=True, stop=False)
                nc.tensor.matmul(out=lgT_ps[:], lhsT=gm_v[:, 0:P],
                                 rhs=h2f_part[:, 3:4], start=False, stop=True)
                stg = sb.tile([P, 3], f32, tag="stg")
                nc.vector.tensor_copy(out=stg[:, 0:1], in_=lgT_ps[:])
                nc.vector.tensor_copy(out=stg[0:v1p, 1:3], in_=h2f_ps[:, 0:2])
                nc.sync.dma_start(
                    out=cc_in[0:1, :].rearrange("a (p w) -> (a p) w", p=P),
                    in_=stg[:])
            else:
                lg2_ps = ps.tile([1, s1p], f32, tag="mm_a", name="lg2")
                nc.tensor.matmul(out=lg2_ps[:], lhsT=h2f_part[:, 2:3],
                                 rhs=g_v, start=True, stop=False)
                nc.tensor.matmul(out=lg2_ps[:], lhsT=h2f_part[:, 3:4],
                                 rhs=gm_v, start=False, stop=True)
                lg2_sb = sb.tile([1, s1p], f32, tag="lg2_sb")
                nc.vector.tensor_copy(out=lg2_sb[:], in_=lg2_ps[:])
                nc.sync.dma_start(
                    out=cc_in[0:1, 0:2 * v1p].rearrange("a (v f) -> (a v) f",
                                                        v=v1p),
                    in_=h2f_part[:, 0:2])
                nc.sync.dma_start(out=cc_in[0:1, 2 * v1p:ccw], in_=lg2_sb[:])
            nc.gpsimd.collective_compute(
                "AllGather", mybir.AluOpType.bypass,
                replica_groups=[list(range(NCORES))],
                ins=[cc_in.opt()], outs=[cc_out.opt()])
            ccg8 = sb.tile([NCORES, ccw], f32, tag="ccg8")
            nc.sync.dma_start(
                out=ccg8[:],
                in_=cc_out[0:1, :].rearrange("a (r w) -> (a r) w", r=NCORES))
            red_ps = ps.tile([1, ccw], f32, tag="mm_a", name="red_ps")
            nc.tensor.matmul(out=red_ps[:], lhsT=ones_v, rhs=ccg8[:],
                             start=True, stop=True)

            # ---- layer-2 softmax at mask node (redundant on all cores) ----
            s1n, v1n = meta["s1n"], meta["v1n"]
            if packed:
                raw2 = red_ps[:].rearrange("a (p w) -> a w p", w=3)[:, 0, :]
                h2view_src = red_ps[:].rearrange(
                    "a (p w) -> a w p", w=3)[:, 1:3, 0:v1n]
            else:
                raw2 = red_ps[:, 2 * v1p:ccw]
                h2view_src = red_ps[:, 0:2 * v1p].rearrange(
                    "a (v f) -> a f v", f=2)[:, :, 0:v1n]
            al2w = P if packed else s1p
            al2t = sb.tile([1, al2w], f32, tag="al2t")
            tmp2 = sb.tile([1, al2w], f32, tag="tmp2")
            nc.vector.tensor_scalar_mul(out=tmp2[:], in0=raw2, scalar1=0.2)
            nc.vector.tensor_tensor(out=al2t[:], in0=raw2, in1=tmp2[:],
                                    op=mybir.AluOpType.max)
            al2 = al2t[:]
            nmx2 = sb.tile([1, 1], f32, tag="nmx2")
            nc.vector.reduce_max(out=nmx2[:], in_=al2[:, 0:s1n],
                                 axis=mybir.AxisListType.X, negate=True)
            nc.scalar.activation(out=al2[:, 0:s1n], in_=al2[:, 0:s1n],
                                 func=mybir.ActivationFunctionType.Exp,
                                 bias=nmx2[:, 0:1])
            sm2 = sb.tile([1, 1], f32, tag="sm2")
            nc.vector.reduce_sum(out=sm2[:], in_=al2[:, 0:s1n],
                                 axis=mybir.AxisListType.X)

            res_sb = sb.tile([1, 2], f32, tag="res_sb")
            if meta["s1_ident"]:
                # sources unique -> alpha2 aligns with V1 rows directly
                wb = al2[:, 0:v1n].rearrange(
                    "a (o v) -> a o v", o=1).to_broadcast([1, 2, v1n])
                prod = sb.tile([1, 2, v1n], f32, tag="prod")
                nc.vector.tensor_tensor(out=prod[:], in0=wb, in1=h2view_src,
                                        op=mybir.AluOpType.mult)
                nc.vector.reduce_sum(out=res_sb[:], in_=prod[:],
                                     axis=mybir.AxisListType.X)
                # normalize by the softmax denominator
                rc2 = sb.tile([1, 1], f32, tag="rc2")
                nc.vector.reciprocal(out=rc2[:], in_=sm2[:])
                nc.vector.tensor_scalar_mul(out=res_sb[:], in0=res_sb[:],
                                            scalar1=rc2[:])
            else:
                # general path: w = (GT @ alpha2^T) / denom, out = w.T @ h2f
                w_ps = ps.tile([1, v1p], f32, tag="mm_b", name="w_ps")
                for k in range(n_s1t):
                    a2T = ps.tile([P, 1], f32, tag="tp", bufs=2, name="a2T")
                    nc.tensor.transpose(out=a2T[:],
                                        in_=al2[:, k * P:(k + 1) * P],
                                        identity=ident[:1, :1])
                    a2Ts = sb.tile([P, 1], f32, tag="a2Ts")
                    nc.vector.tensor_copy(out=a2Ts[:], in_=a2T[:])
                    nc.tensor.matmul(out=w_ps[:], lhsT=a2Ts[:],
                                     rhs=gt_v[:, k, :],
                                     start=(k == 0), stop=(k == n_s1t - 1))
                rc2 = sb.tile([1, 1], f32, tag="rc2")
                nc.vector.reciprocal(out=rc2[:], in_=sm2[:])
                w_row = sb.tile([1, v1p], f32, tag="w_row")
                nc.vector.tensor_scalar_mul(out=w_row[:], in0=w_ps[:],
                                            scalar1=rc2[:])
                wb = w_row[:, 0:v1n].rearrange(
                    "a (o v) -> a o v", o=1).to_broadcast([1, 2, v1n])
                prod2 = sb.tile([1, 2, v1n], f32, tag="prod2")
                nc.vector.tensor_tensor(out=prod2[:], in0=wb, in1=h2view_src,
                                        op=mybir.AluOpType.mult)
                nc.vector.reduce_sum(out=res_sb[:], in_=prod2[:],
                                     axis=mybir.AxisListType.X)

            nc.vector.tensor_add(out=res_sb[:], in0=res_sb[:], in1=oxm_sb[:])
            nc.sync.dma_start(out=d_res[:], in_=res_sb[:])

    nc.compile()
    return nc


_CACHE = {}


def _get_nc(meta):
    key = repr(sorted(meta.items()))
    if key not in _CACHE:
        _CACHE[key] = _build(meta)
    return _CACHE[key]


def make_in_maps(**inputs):
    """Host preprocessing: shard/fold inputs into per-core input maps."""
    x = np.asarray(inputs["x"], np.float32)
    n_nodes = x.shape[0]
    meta, host = _preprocess(inputs["edge_index"], inputs["mask_idx"], n_nodes)

    W1 = np.asarray(inputs["W1"], np.float32)
    att_s1 = np.asarray(inputs["att_src1"], np.float32)
    att_d1 = np.asarray(inputs["att_dst1"], np.float32)
    b1 = np.asarray(inputs["b1"], np.float32)
    W2 = np.asarray(inputs["W2"], np.float32)
    att_s2 = np.asarray(inputs["att_src2"], np.float32)
    att_d2 = np.asarray(inputs["att_dst2"], np.float32)
    b2 = np.asarray(inputs["b2"], np.float32)
    fc_w = np.asarray(inputs["fc_w"], np.float32)
    fc_b = np.asarray(inputs["fc_b"], np.float32)
    cls_w = np.asarray(inputs["cls_w"], np.float32)
    cls_b = np.asarray(inputs["cls_b"], np.float32)

    Ws1 = np.einsum("chf,hf->ch", W1.reshape(C, H1, OUT), att_s1)  # [C, H1]
    Wd1 = np.einsum("chf,hf->ch", W1.reshape(C, H1, OUT), att_d1)
    Ws2 = W2 @ att_s2[0]                                           # [H1*OUT]
    Wd2 = W2 @ att_d2[0]
    # classifier fold: out = cat @ fc_w @ cls_w + (fc_b @ cls_w + cls_b)
    wf = fc_w @ cls_w                                              # [1536, 2]
    wf_top, wf_bot = wf[:OUT], wf[OUT:]
    w2fold = W2 @ wf_top                                           # [6144, 2]
    bias3 = (b2 @ wf_top + fc_b @ cls_w + cls_b).reshape(1, 2).astype(np.float32)

    n_s1t, v1p, s1p = meta["n_s1t"], meta["v1p"], meta["s1p"]
    n_et = meta["n_et"]
    s2p = n_et * P
    gt_pad = np.zeros((n_s1t * P, v1p), np.float32)
    gt_pad[:s1p] = host["gt"]

    # pre-gathered + pre-transposed x rows (index-select = sharding)
    s2p_ = meta["n_et"] * P
    xg = x[host["src_ids"]]                                        # [s2p, 768]
    xgt3 = np.ascontiguousarray(xg.T).reshape(KC, P, s2p_)
    ws13 = Ws1.reshape(KC, P, H1)
    xga = np.concatenate([xgt3, ws13], axis=2)                     # [KC,128,s2p+8]
    xga = np.ascontiguousarray(
        xga.transpose(1, 0, 2).reshape(P, KC * (s2p_ + H1)))
    xv = x[host["v1_ids"]]                                         # [v1p, 768]
    xvt = _chunked(np.ascontiguousarray(xv.T))                     # [128, KC*v1p]
    ones8 = np.ones((NCORES, 1), np.float32)

    lay, cw = _const_layout(meta)

    def fill(cst, name, arr):
        rows, off, cols = lay[name]
        assert arr.shape == (rows, cols), (name, arr.shape, (rows, cols))
        cst[0:rows, off:off + cols] = arr

    m01_pack = np.concatenate(
        [host["m01"][t * P:(t + 1) * P] for t in range(n_et)], axis=1)

    in_maps = []
    for i in range(NCORES):
        w1blk = np.ascontiguousarray(W1[:, i * OUT:(i + 1) * OUT])
        w2fblk = np.concatenate(
            [w2fold[i * OUT:(i + 1) * OUT, :],
             Ws2[i * OUT:(i + 1) * OUT, None],
             Wd2[i * OUT:(i + 1) * OUT, None]], axis=1)            # [768, 4]
        head = np.zeros((H1, 1), np.float32)
        head[i % H1, 0] = 1.0
        cst = np.zeros((P, cw), np.float32)
        fill(cst, "xvt", xvt)
        fill(cst, "ones", ones8)
        fill(cst, "wd1", _chunked(Wd1))
        fill(cst, "w2f", _chunked(w2fblk))
        fill(cst, "wfb", _chunked(np.ascontiguousarray(wf_bot)))
        fill(cst, "b1", _colmajor(b1[i * OUT:(i + 1) * OUT]))
        fill(cst, "xm", _colmajor(np.ascontiguousarray(x[meta["m"]])))
        fill(cst, "m01", m01_pack)
        fill(cst, "m01t", host["m01t"])
        fill(cst, "g", host["g"])
        fill(cst, "gm", host["gm"])
        fill(cst, "gt", _chunked(gt_pad))
        fill(cst, "padbias", host["padbias"])
        fill(cst, "bias3", bias3)
        fill(cst, "head", head)
        im = {
            "xga": xga,
            "cst": cst,
        }
        for c in range(KC):
            im[f"w1c{c}"] = np.ascontiguousarray(w1blk[c * P:(c + 1) * P, :])
        in_maps.append(im)
    return meta, in_maps


def kernel(**inputs):
    meta, in_maps = make_in_maps(**inputs)
    nc = _get_nc(meta)
    res = run_bass_kernel_spmd(nc, in_maps, core_ids=list(range(NCORES)))
    return res.results[0]["res"].astype(np.float32)



# revision 6
# speedup vs baseline: 1.5425x; 1.5425x over previous
"""Trainium2 Bass kernel for the 2-layer GAT node-classification head.

The reference reads only h2[mask_idx] and x[mask_idx] for the classifier, so
the exact computation collapses to mask_idx's 2-hop in-neighborhood:

  V1 = sources of mask's in-edges (incl. the self-loop), S2 = in-edges of V1,
  U  = unique sources of S2.  |V1|=2, |S2|=7, |U|=6 for this graph.

Per-core plan (identical on all 8 cores -- the cost model charges a flat
15us constant for ANY collective, which dwarfs the whole problem, so the
fastest distribution is full replication with zero communication):

  1. attention: a_src/a_dst at U via folded Ws1/Wd1 (one-hot scatter to the
     edge layout), segment softmax without max-shift (logits are tiny), all
     heads at once.
  2. aggregate-first: since the value aggregation is linear in x, build
     per-(head, dst) weighted x sums (xagg) BEFORE the big GEMM; the
     [768 x 6144] W1 GEMM then has only v1n output columns per head.
  3. W1 streams in fp8 (x64 prescale to clear the e4m3 subnormal range) in
     6 chunk DMAs pipelined against the PSUM-accumulating GEMM.  DMA bytes
     dominate the kernel; fp8 quarters them vs f32.
  4. elu via exp(min(x,0)) = min(exp(x),1); the "-1" of elu folds into host
     constants.  Layer-2 logits/softmax and the classifier fold into a
     [6144, 4] bf16 contraction + tiny fixed tail.

Host preprocessing: graph cone extraction + one-hot scatter matrices
(index-select = sharding) and weight-weight folds (W1@att, W2@fold), as in
the original head-sharded version.
"""

import numpy as np
import ml_dtypes

import concourse.bass as bass
import concourse.mybir as mybir
import concourse.tile as tile
from concourse import bacc
from concourse.bass_utils import run_bass_kernel_spmd
from concourse.masks import make_identity

NCORES = 8
P = 128
C = 768          # input feature dim
H1 = 8           # layer-1 heads
OUT = 768        # per-head feature dim
KC = C // P      # 6 k-chunks of 128 over the 768 contraction
NEGPAD = -745.0  # padding logit: exp(0.2 * NEGPAD) == 0 in f32
W1SCALE = 64.0   # fp8 prescale for W1 (clears e4m3 subnormals)

f32 = mybir.dt.float32
bf16 = mybir.dt.bfloat16
fp8 = mybir.dt.float8e4
np_bf16 = ml_dtypes.bfloat16
np_fp8 = ml_dtypes.float8_e4m3


# ---------------------------------------------------------------- host graph
def _preprocess(edge_index, mask_idx, n_nodes):
    """Extract the 2-hop in-neighborhood of mask_idx. meta is compile-time
    (shapes only); host holds the data (one-hot matrices, index lists)."""
    ei = np.asarray(edge_index).astype(np.int64)
    m = int(np.asarray(mask_idx))
    src_all = np.concatenate([ei[0], np.arange(n_nodes, dtype=np.int64)])
    dst_all = np.concatenate([ei[1], np.arange(n_nodes, dtype=np.int64)])

    s1_pos = np.nonzero(dst_all == m)[0]          # in-edges of m (incl self)
    s1_src = src_all[s1_pos].tolist()
    s1n = len(s1_src)
    v1 = list(dict.fromkeys(s1_src))              # unique sources
    v1n = len(v1)
    assert v1n <= 8, f"mask in-degree too large for this layout: {v1n}"

    groups = [src_all[np.nonzero(dst_all == v)[0]].tolist() for v in v1]
    gmax = max(len(g) for g in groups)
    s2p = v1n * gmax
    assert s2p <= P, f"edge tile too large: {s2p}"

    u = list(dict.fromkeys([s for g in groups for s in g]))
    un = len(u)
    up = 16
    while up < un:
        up *= 2
    assert v1n * up <= P, f"wuv tile too large: {v1n * up}"
    urow = {node: r for r, node in enumerate(u)}

    # S2 edge slot layout: group g occupies cols [g*gmax, g*gmax+len(g))
    u2e = np.zeros((up, s2p), np.float32)         # src scatter
    d2e = np.zeros((up, s2p), np.float32)         # dst scatter
    pad01 = np.zeros((1, s2p), np.float32)
    sv01 = np.zeros((s2p, v1n * up), np.float32)  # edge -> (v,u) accumulate
    for g, srcs in enumerate(groups):
        for j in range(gmax):
            e = g * gmax + j
            if j < len(srcs):
                su = urow[srcs[j]]
                u2e[su, e] = 1.0
                d2e[urow[v1[g]], e] = 1.0
                sv01[e, g * up + su] = 1.0
            else:
                pad01[0, e] = 1.0

    # layer-2 (s1) structure
    v1row = {v: r for r, v in enumerate(v1)}
    g_mat = np.zeros((v1n, s1n), np.float32)
    gm_mat = np.zeros((v1n, s1n), np.float32)
    for e, s in enumerate(s1_src):
        g_mat[v1row[s], e] = 1.0
        gm_mat[v1row[m], e] = 1.0
    s1_ident = (s1n == v1n) and all(v1row[s] == e for e, s in enumerate(s1_src))

    meta = dict(v1n=v1n, s1n=s1n, gmax=gmax, un=un, up=up, s1_ident=s1_ident)
    host = dict(m=m, v1=v1, u=u, u2e=u2e, d2e=d2e, pad01=pad01, sv01=sv01,
                g=g_mat, gm=gm_mat)
    return meta, host


def _lay16(meta):
    """Column layout of the bf16 packed-constants tensor."""
    up, s2p = meta["up"], meta["v1n"] * meta["gmax"]
    pieces = [
        ("xut", P, KC * up),        # x[U]^T chunked  [128, KC*up]
        ("wsd1", P, KC * 2 * H1),   # [Ws1|Wd1] chunked
        ("u2e", up, s2p),
        ("d2e", up, s2p),
        ("pad01", 1, s2p),
        ("neg8", 1, H1),
        ("sv01", s2p, meta["v1n"] * up),
    ]
    lay, off = {}, 0
    for name, rows, cols in pieces:
        lay[name] = (rows, off, cols)
        off += cols
    return lay, off


def _lay32(meta):
    """Column layout of the f32 packed-constants tensor (tail/oxm)."""
    v1n, s1n = meta["v1n"], meta["s1n"]
    pieces = [
        ("xm", P, KC),
        ("wfb", P, KC * 2),
        ("g", v1n, s1n),
        ("gm", v1n, s1n),
        ("shiftrow", 1, s1n),
        ("one11", 1, 1),
        ("bias3s", 1, 2),
        ("ones_s1", s1n, 1),
    ]
    lay, off = {}, 0
    for name, rows, cols in pieces:
        lay[name] = (rows, off, cols)
        off += cols
    return lay, off


def _chunked(w):
    """[K, N] -> [128, (K//128)*N] chunk-major free layout."""
    k, n = w.shape
    assert k % P == 0
    return np.ascontiguousarray(
        w.reshape(k // P, P, n).transpose(1, 0, 2).reshape(P, (k // P) * n))


# ---------------------------------------------------------------- bass build
def _build(meta):
    v1n, s1n, gmax = meta["v1n"], meta["s1n"], meta["gmax"]
    up, s1_ident = meta["up"], meta["s1_ident"]
    s2p = v1n * gmax
    nblk = H1 * KC                  # 48 (head, f-chunk) output blocks
    lay16, cw16 = _lay16(meta)
    lay32, cw32 = _lay32(meta)

    nc = bacc.Bacc("TRN2", target_bir_lowering=False, debug=False,
                   enable_asserts=True, num_devices=NCORES)

    d_cst16 = nc.dram_tensor("cst16", [P, cw16], bf16, kind="ExternalInput")
    d_xu = nc.dram_tensor("xu", [up, C], bf16, kind="ExternalInput")
    d_w1b = nc.dram_tensor("w1b", [1, H1 * OUT], fp8, kind="ExternalInput")
    d_w1 = [nc.dram_tensor(f"w1c{c}", [P, H1 * OUT], fp8, kind="ExternalInput")
            for c in range(KC)]
    d_w2f = nc.dram_tensor("w2f", [P, nblk * 4], bf16, kind="ExternalInput")
    d_cst32 = nc.dram_tensor("cst32", [P, cw32], f32, kind="ExternalInput")
    d_res = nc.dram_tensor("res", [1, 2], f32, kind="ExternalOutput")

    with tile.TileContext(nc) as tc:
        with (
            tc.tile_pool(name="const", bufs=1) as cpool,
            tc.tile_pool(name="sbuf", bufs=1) as sb,
            tc.tile_pool(name="big", bufs=1) as bigp,
            tc.tile_pool(name="ps", bufs=1, space="PSUM") as ps,
        ):
            # ---- input DMAs: attention inputs first, W1 stream, tail last
            cst16 = cpool.tile([P, cw16], bf16, tag="cst16")
            nc.sync.dma_start(out=cst16[:], in_=d_cst16[:])
            xu_sb = cpool.tile([up, C], bf16, tag="xu")
            nc.sync.dma_start(out=xu_sb[:], in_=d_xu[:])
            w1b_sb = cpool.tile([1, H1 * OUT], fp8, tag="w1b")
            nc.sync.dma_start(out=w1b_sb[:], in_=d_w1b[:])
            w1_sb = [bigp.tile([P, H1 * OUT], fp8, tag=f"w1_{c}",
                               name=f"w1_{c}") for c in range(KC)]
            for c in range(KC):
                nc.sync.dma_start(out=w1_sb[c][:], in_=d_w1[c][:])
            w2f_sb = cpool.tile([P, nblk * 4], bf16, tag="w2f")
            nc.sync.dma_start(out=w2f_sb[:], in_=d_w2f[:])
            cst32 = cpool.tile([P, cw32], f32, tag="cst32")
            nc.sync.dma_start(out=cst32[:], in_=d_cst32[:])

            def cv16(name):
                rows, off, cols = lay16[name]
                return cst16[0:rows, off:off + cols]

            def cv32(name):
                rows, off, cols = lay32[name]
                return cst32[0:rows, off:off + cols]

            xut_v = cv16("xut").rearrange("p (k n) -> p k n", k=KC)
            wsd1_v = cv16("wsd1").rearrange("p (k n) -> p k n", k=KC)
            u2e_v = cv16("u2e")
            d2e_v = cv16("d2e")
            pad01_v = cv16("pad01")
            neg8_v = cv16("neg8")
            sv01_v = cv16("sv01")

            ident = cpool.tile([H1, H1], f32, tag="ident")
            make_identity(nc, ident[:])
            ones2 = cpool.tile([1, v1n], fp8, tag="ones2")
            nc.vector.memset(ones2[:], 1.0)

            # ---- attention: a_src/a_dst at U, all heads ----
            attb = ps.tile([P, 512], f32, tag="attbank")
            asd_ps = attb[0:up, 0:2 * H1]
            lg_ps = attb[0:H1, 16:16 + s2p]
            at_ps = attb[0:s2p, 144:144 + H1]
            wuv_ps = [attb[0:up, 152 + 8 * v:160 + 8 * v]
                      for v in range(v1n)]
            for c in range(KC):
                nc.tensor.matmul(out=asd_ps, lhsT=xut_v[:, c, :],
                                 rhs=wsd1_v[:, c, :],
                                 start=(c == 0), stop=(c == KC - 1))
            asd_sb = sb.tile([up, 2 * H1], bf16, tag="asd_sb")
            nc.vector.tensor_copy(out=asd_sb[:], in_=asd_ps)

            # per-edge logits: a_s[src_e] + a_d[dst_e] + pad bias
            nc.tensor.matmul(out=lg_ps, lhsT=asd_sb[:, 0:H1], rhs=u2e_v,
                             start=True, stop=False)
            nc.tensor.matmul(out=lg_ps, lhsT=asd_sb[:, H1:2 * H1],
                             rhs=d2e_v, start=False, stop=False)
            nc.tensor.matmul(out=lg_ps, lhsT=neg8_v, rhs=pad01_v,
                             start=False, stop=True)

            # leaky-relu (one Act op), exp without max-shift (logits tiny),
            # then per-group normalize
            lg_sb = sb.tile([H1, s2p], f32, tag="lg_sb")
            nc.scalar.activation(out=lg_sb[:], in_=lg_ps,
                                 func=mybir.ActivationFunctionType.Lrelu,
                                 alpha=0.2)
            ee_sb = sb.tile([H1, s2p], f32, tag="ee_sb")
            nc.scalar.activation(out=ee_sb[:], in_=lg_sb[:],
                                 func=mybir.ActivationFunctionType.Exp)
            eev = ee_sb[:].rearrange("h (g e) -> h g e", e=gmax)
            den = sb.tile([H1, v1n], f32, tag="den")
            nc.vector.reduce_sum(out=den[:], in_=eev,
                                 axis=mybir.AxisListType.X)
            rec = sb.tile([H1, v1n], f32, tag="rec")
            nc.vector.reciprocal(out=rec[:], in_=den[:])
            alpha_sb = sb.tile([H1, s2p], f32, tag="alpha_sb")
            recb = rec[:].rearrange("h (g o) -> h g o", o=1).to_broadcast(
                [H1, v1n, gmax])
            nc.vector.tensor_tensor(
                out=alpha_sb[:].rearrange("h (g e) -> h g e", e=gmax),
                in0=eev, in1=recb, op=mybir.AluOpType.mult)

            # alpha^T via PE transpose, then wuv[(v,u), h] = sum_e alpha
            nc.tensor.transpose(out=at_ps, in_=alpha_sb[:],
                                identity=ident[:])
            at_sb = sb.tile([s2p, H1], bf16, tag="at_sb")
            nc.vector.tensor_copy(out=at_sb[:], in_=at_ps)
            # per-v blocks: PE/DVE partition bases must be 0/32/64-aligned
            wuv_sb = [sb.tile([up, H1], bf16, tag=f"wuv_sb{v}",
                              name=f"wuv_sb{v}") for v in range(v1n)]
            for v in range(v1n):
                nc.tensor.matmul(out=wuv_ps[v],
                                 lhsT=sv01_v[:, v * up:(v + 1) * up],
                                 rhs=at_sb[:], start=True, stop=True)
                nc.vector.tensor_copy(out=wuv_sb[v][:], in_=wuv_ps[v])

            # xagg^T chunks: [128c, (c,v,h)] = sum_u x[U]^T wuv
            xagg_ps = ps.tile([P, KC * v1n * H1], f32, tag="xagg")
            for c in range(KC):
                for v in range(v1n):
                    nc.tensor.matmul(
                        out=xagg_ps[:, (c * v1n + v) * H1:
                                    (c * v1n + v + 1) * H1],
                        lhsT=xu_sb[:, c * P:(c + 1) * P],
                        rhs=wuv_sb[v][:],
                        start=True, stop=True)
            xagg8 = sb.tile([P, KC * v1n * H1], fp8, tag="xagg8")
            nc.vector.tensor_copy(out=xagg8[:], in_=xagg_ps[:])
            xagg8_v = xagg8[:].rearrange("p (c v h) -> p c v h", c=KC, v=v1n)

            # ---- the big GEMM: agg[f, (h,fc,v)] = xagg @ (64*W1) + 64*b1
            # bias row first (start), then one accumulate pass per W1 chunk
            # as its DMA lands; fp8 x fp8 -> f32 PSUM.
            agg_ps = ps.tile([P, nblk * v1n], f32, tag="agg")
            for k in range(nblk):
                h, fc = divmod(k, KC)
                nc.tensor.matmul(
                    out=agg_ps[:, k * v1n:(k + 1) * v1n],
                    lhsT=w1b_sb[0:1, h * OUT + fc * P:h * OUT + (fc + 1) * P],
                    rhs=ones2[:], start=True, stop=False,
                    skip_group_check=True)
            for c in range(KC):
                for k in range(nblk):
                    h, fc = divmod(k, KC)
                    nc.tensor.matmul(
                        out=agg_ps[:, k * v1n:(k + 1) * v1n],
                        lhsT=w1_sb[c][:, h * OUT + fc * P:
                                      h * OUT + (fc + 1) * P],
                        rhs=xagg8_v[:, c, :, h],
                        start=False, stop=(c == KC - 1),
                        skip_group_check=True)

            # elu'(x) = elu(x) + 1 = max(x,0) + min(exp(x),1); x = agg/64.
            # The -1 is folded into host constants downstream.
            t1_sb = sb.tile([P, nblk * v1n], f32, tag="t1_sb")
            nc.vector.tensor_scalar(out=t1_sb[:], in0=agg_ps[:],
                                    scalar1=1.0 / W1SCALE, scalar2=0.0,
                                    op0=mybir.AluOpType.mult,
                                    op1=mybir.AluOpType.max)
            ee2_sb = sb.tile([P, nblk * v1n], f32, tag="ee2_sb")
            nc.scalar.activation(out=ee2_sb[:], in_=agg_ps[:],
                                 func=mybir.ActivationFunctionType.Exp,
                                 scale=1.0 / W1SCALE)
            nc.vector.tensor_scalar(out=ee2_sb[:], in0=ee2_sb[:],
                                    scalar1=1.0, scalar2=None,
                                    op0=mybir.AluOpType.min)
            helu_sb = sb.tile([P, nblk * v1n], bf16, tag="helu_sb")
            nc.vector.tensor_tensor(out=helu_sb[:], in0=t1_sb[:],
                                    in1=ee2_sb[:], op=mybir.AluOpType.add)

            # ---- oxm = x[m] @ wf_bot + bias3s (off critical path) ----
            xm_v = cv32("xm")
            wfb_v = cv32("wfb").rearrange("p (k n) -> p k n", k=KC)
            one11_v = cv32("one11")
            bias3s_v = cv32("bias3s")
            tailb = ps.tile([P, 12], f32, tag="tailbank")
            oxm_ps = tailb[0:1, 0:2]
            h2f_ps = tailb[0:v1n, 2:6]
            r2t_ps = tailb[0:s1n, 6:7]
            den_ps = tailb[0:1, 7:8]
            fin_ps = tailb[0:1, 8:10]
            for c in range(KC):
                nc.tensor.matmul(out=oxm_ps, lhsT=xm_v[:, c:c + 1],
                                 rhs=wfb_v[:, c, :],
                                 start=(c == 0), stop=False)
            nc.tensor.matmul(out=oxm_ps, lhsT=one11_v, rhs=bias3s_v,
                             start=False, stop=True)

            # ---- folded layer-2: h2f'[v, 0:4] = helu' @ [w2fold|Ws2|Wd2]
            for k in range(nblk):
                nc.tensor.matmul(out=h2f_ps,
                                 lhsT=helu_sb[:, k * v1n:(k + 1) * v1n],
                                 rhs=w2f_sb[:, k * 4:(k + 1) * 4],
                                 start=(k == 0), stop=(k == nblk - 1))
            h2f_sb = sb.tile([v1n, 4], f32, tag="h2f_sb")
            nc.vector.tensor_copy(out=h2f_sb[:], in_=h2f_ps)

            # ---- layer-2 logits (transposed), softmax, weighted sum ----
            g_v = cv32("g")
            gm_v = cv32("gm")
            shiftrow_v = cv32("shiftrow")
            ones_s1_v = cv32("ones_s1")
            nc.tensor.matmul(out=r2t_ps, lhsT=g_v, rhs=h2f_sb[:, 2:3],
                             start=True, stop=False)
            nc.tensor.matmul(out=r2t_ps, lhsT=gm_v, rhs=h2f_sb[:, 3:4],
                             start=False, stop=False)
            nc.tensor.matmul(out=r2t_ps, lhsT=shiftrow_v, rhs=one11_v,
                             start=False, stop=True)
            al2t_sb = sb.tile([s1n, 1], f32, tag="al2t_sb")
            nc.scalar.activation(out=al2t_sb[:], in_=r2t_ps,
                                 func=mybir.ActivationFunctionType.Lrelu,
                                 alpha=0.2)
            e2t_sb = sb.tile([s1n, 1], f32, tag="e2t_sb")
            nc.scalar.activation(out=e2t_sb[:], in_=al2t_sb[:],
                                 func=mybir.ActivationFunctionType.Exp)

            nc.tensor.matmul(out=den_ps, lhsT=e2t_sb[:], rhs=ones_s1_v,
                             start=True, stop=True)
            if s1_ident:
                nc.tensor.matmul(out=fin_ps, lhsT=e2t_sb[:],
                                 rhs=h2f_sb[:, 0:2], start=True, stop=True)
            else:
                gath_ps = tailb[0:s1n, 10:12]
                nc.tensor.matmul(out=gath_ps, lhsT=g_v,
                                 rhs=h2f_sb[:, 0:2], start=True, stop=True)
                gath_sb = sb.tile([s1n, 2], f32, tag="gath_sb")
                nc.vector.tensor_copy(out=gath_sb[:], in_=gath_ps)
                nc.tensor.matmul(out=fin_ps, lhsT=e2t_sb[:],
                                 rhs=gath_sb[:], start=True, stop=True)

            rec2 = sb.tile([1, 1], f32, tag="rec2")
            nc.vector.reciprocal(out=rec2[:], in_=den_ps)
            res1 = sb.tile([1, 2], f32, tag="res1")
            nc.vector.tensor_scalar(out=res1[:], in0=fin_ps,
                                    scalar1=rec2[:, 0:1], scalar2=None,
                                    op0=mybir.AluOpType.mult)
            res_sb = sb.tile([1, 2], f32, tag="res_sb")
            nc.vector.tensor_add(out=res_sb[:], in0=res1[:],
                                 in1=oxm_ps)
            nc.sync.dma_start(out=d_res[:], in_=res_sb[:])

    nc.compile()
    return nc


_CACHE = {}


def _get_nc(meta):
    key = repr(sorted(meta.items()))
    if key not in _CACHE:
        _CACHE[key] = _build(meta)
    return _CACHE[key]


def make_in_maps(**inputs):
    x = np.asarray(inputs["x"], np.float32)
    n_nodes = x.shape[0]
    meta, host = _preprocess(inputs["edge_index"], inputs["mask_idx"], n_nodes)
    v1n, s1n, up = meta["v1n"], meta["s1n"], meta["up"]
    s2p = v1n * meta["gmax"]
    nblk = H1 * KC

    W1 = np.asarray(inputs["W1"], np.float32)
    att_s1 = np.asarray(inputs["att_src1"], np.float32)
    att_d1 = np.asarray(inputs["att_dst1"], np.float32)
    b1 = np.asarray(inputs["b1"], np.float32)
    W2 = np.asarray(inputs["W2"], np.float32)
    att_s2 = np.asarray(inputs["att_src2"], np.float32)
    att_d2 = np.asarray(inputs["att_dst2"], np.float32)
    b2 = np.asarray(inputs["b2"], np.float32)
    fc_w = np.asarray(inputs["fc_w"], np.float32)
    fc_b = np.asarray(inputs["fc_b"], np.float32)
    cls_w = np.asarray(inputs["cls_w"], np.float32)
    cls_b = np.asarray(inputs["cls_b"], np.float32)

    # weight-weight folds
    Ws1 = np.einsum("chf,hf->ch", W1.reshape(C, H1, OUT), att_s1)   # [C, H1]
    Wd1 = np.einsum("chf,hf->ch", W1.reshape(C, H1, OUT), att_d1)
    Ws2 = W2 @ att_s2[0]                                            # [H1*OUT]
    Wd2 = W2 @ att_d2[0]
    wf = fc_w @ cls_w                                               # [1536, 2]
    wf_top, wf_bot = wf[:OUT], wf[OUT:]
    w2fold = W2 @ wf_top                                            # [6144, 2]
    # helu' = elu + 1 fold: subtract column sums; softmax shift constant
    shift_const = -(Ws2.sum() + Wd2.sum())
    bias3s = (b2 @ wf_top + fc_b @ cls_w + cls_b
              - w2fold.sum(axis=0)).reshape(1, 2).astype(np.float32)

    # w2f blocks ordered to match agg blocks k = h*KC + fc
    w2f4 = np.concatenate([w2fold, Ws2[:, None], Wd2[:, None]], axis=1)
    w2f_host = np.zeros((P, nblk * 4), np.float32)
    for k in range(nblk):
        w2f_host[:, k * 4:(k + 1) * 4] = w2f4[k * P:(k + 1) * P, :]

    # bf16 constants tensor
    lay16, cw16 = _lay16(meta)
    cst16 = np.zeros((P, cw16), np.float32)

    def fill16(name, arr):
        rows, off, cols = lay16[name]
        assert arr.shape == (rows, cols), (name, arr.shape, (rows, cols))
        cst16[0:rows, off:off + cols] = arr

    xu_rows = np.zeros((up, C), np.float32)
    xu_rows[:meta["un"]] = x[host["u"]]
    xut = np.zeros((P, KC * up), np.float32)
    for c in range(KC):
        xut[:, c * up:(c + 1) * up] = xu_rows[:, c * P:(c + 1) * P].T
    fill16("xut", xut)
    fill16("wsd1", _chunked(np.concatenate([Ws1, Wd1], axis=1)))
    fill16("u2e", host["u2e"])
    fill16("d2e", host["d2e"])
    fill16("pad01", host["pad01"])
    fill16("neg8", np.full((1, H1), NEGPAD, np.float32))
    fill16("sv01", host["sv01"])

    # f32 constants tensor (tail)
    lay32, cw32 = _lay32(meta)
    cst32 = np.zeros((P, cw32), np.float32)

    def fill32(name, arr):
        rows, off, cols = lay32[name]
        assert arr.shape == (rows, cols), (name, arr.shape, (rows, cols))
        cst32[0:rows, off:off + cols] = arr

    fill32("xm", np.ascontiguousarray(x[host["m"]].reshape(KC, P).T))
    fill32("wfb", _chunked(np.ascontiguousarray(wf_bot)))
    fill32("g", host["g"])
    fill32("gm", host["gm"])
    fill32("shiftrow", np.full((1, s1n), shift_const, np.float32))
    fill32("one11", np.ones((1, 1), np.float32))
    fill32("bias3s", bias3s)
    fill32("ones_s1", np.ones((s1n, 1), np.float32))

    w1s = (W1 * W1SCALE).astype(np_fp8)                 # [768, 6144] fp8
    w1b = (b1 * W1SCALE).astype(np_fp8).reshape(1, H1 * OUT)

    im = {
        "cst16": cst16.astype(np_bf16),
        "xu": xu_rows.astype(np_bf16),
        "w1b": w1b,
        "w2f": w2f_host.astype(np_bf16),
        "cst32": cst32,
    }
    for c in range(KC):
        im[f"w1c{c}"] = np.ascontiguousarray(w1s[c * P:(c + 1) * P, :])
    return meta, [im] * NCORES


def kernel(**inputs):
    meta, in_maps = make_in_maps(**inputs)
    nc = _get_nc(meta)
    res = run_bass_kernel_spmd(nc, in_maps, core_ids=list(range(NCORES)))
    return res.results[0]["res"].astype(np.float32)


# revision 7
# speedup vs baseline: 1.8109x; 1.1740x over previous
"""Trainium2 Bass kernel for the 2-layer GAT node-classification head.

The reference reads only h2[mask_idx] and x[mask_idx] for the classifier, so
the exact computation collapses to mask_idx's 2-hop in-neighborhood:

  V1 = sources of mask's in-edges (incl. the self-loop), S2 = in-edges of V1,
  U  = unique sources of S2.  |V1|=2, |S2|=7, |U|=6 for this graph.

Per-core plan (identical on all 8 cores -- the cost model charges a flat
15us constant for ANY collective, which dwarfs the whole problem, so the
fastest distribution is full replication with zero communication):

  1. attention: a_src/a_dst at U via folded Ws1/Wd1 (one-hot scatter to the
     edge layout), segment softmax without max-shift (logits are tiny), all
     heads at once.
  2. aggregate-first: since the value aggregation is linear in x, build
     per-(head, dst) weighted x sums (xagg) BEFORE the big GEMM; the
     [768 x 6144] W1 GEMM then has only v1n output columns per head.
  3. W1 streams in fp8 (x64 prescale to clear the e4m3 subnormal range) in
     6 chunk DMAs pipelined against the PSUM-accumulating GEMM.  DMA bytes
     dominate the kernel; fp8 quarters them vs f32.
  4. elu via exp(min(x,0)) = min(exp(x),1); the "-1" of elu folds into host
     constants.  Layer-2 logits/softmax and the classifier fold into a
     [6144, 4] bf16 contraction + tiny fixed tail.

Host preprocessing: graph cone extraction + one-hot scatter matrices
(index-select = sharding) and weight-weight folds (W1@att, W2@fold), as in
the original head-sharded version.
"""

import numpy as np
import ml_dtypes

import concourse.bass as bass
import concourse.mybir as mybir
import concourse.tile as tile
from concourse import bacc
from concourse.bass_utils import run_bass_kernel_spmd
from concourse.masks import make_identity

NCORES = 8
P = 128
C = 768          # input feature dim
H1 = 8           # layer-1 heads
OUT = 768        # per-head feature dim
KC = C // P      # 6 k-chunks of 128 over the 768 contraction
NEGPAD = -745.0  # padding logit: exp(0.2 * NEGPAD) == 0 in f32
W1SCALE = 64.0   # fp8 prescale for W1 (clears e4m3 subnormals)

f32 = mybir.dt.float32
bf16 = mybir.dt.bfloat16
fp8 = mybir.dt.float8e4
np_bf16 = ml_dtypes.bfloat16
np_fp8 = ml_dtypes.float8_e4m3


# ---------------------------------------------------------------- host graph
def _preprocess(edge_index, mask_idx, n_nodes):
    """Extract the 2-hop in-neighborhood of mask_idx. meta is compile-time
    (shapes only); host holds the data (one-hot matrices, index lists)."""
    ei = np.asarray(edge_index).astype(np.int64)
    m = int(np.asarray(mask_idx))
    src_all = np.concatenate([ei[0], np.arange(n_nodes, dtype=np.int64)])
    dst_all = np.concatenate([ei[1], np.arange(n_nodes, dtype=np.int64)])

    s1_pos = np.nonzero(dst_all == m)[0]          # in-edges of m (incl self)
    s1_src = src_all[s1_pos].tolist()
    s1n = len(s1_src)
    v1 = list(dict.fromkeys(s1_src))              # unique sources
    v1n = len(v1)
    assert v1n <= 8, f"mask in-degree too large for this layout: {v1n}"

    groups = [src_all[np.nonzero(dst_all == v)[0]].tolist() for v in v1]
    gmax = max(len(g) for g in groups)
    s2p = v1n * gmax
    assert s2p <= P, f"edge tile too large: {s2p}"

    u = list(dict.fromkeys([s for g in groups for s in g]))
    un = len(u)
    up = 16
    while up < un:
        up *= 2
    assert v1n * up <= P, f"wuv tile too large: {v1n * up}"
    urow = {node: r for r, node in enumerate(u)}

    # S2 edge slot layout: group g occupies cols [g*gmax, g*gmax+len(g))
    u2e = np.zeros((up, s2p), np.float32)         # src scatter
    d2e = np.zeros((up, s2p), np.float32)         # dst scatter
    pad01 = np.zeros((1, s2p), np.float32)
    sv01 = np.zeros((s2p, v1n * up), np.float32)  # edge -> (v,u) accumulate
    for g, srcs in enumerate(groups):
        for j in range(gmax):
            e = g * gmax + j
            if j < len(srcs):
                su = urow[srcs[j]]
                u2e[su, e] = 1.0
                d2e[urow[v1[g]], e] = 1.0
                sv01[e, g * up + su] = 1.0
            else:
                pad01[0, e] = 1.0

    # layer-2 (s1) structure
    v1row = {v: r for r, v in enumerate(v1)}
    g_mat = np.zeros((v1n, s1n), np.float32)
    gm_mat = np.zeros((v1n, s1n), np.float32)
    for e, s in enumerate(s1_src):
        g_mat[v1row[s], e] = 1.0
        gm_mat[v1row[m], e] = 1.0
    s1_ident = (s1n == v1n) and all(v1row[s] == e for e, s in enumerate(s1_src))

    meta = dict(v1n=v1n, s1n=s1n, gmax=gmax, un=un, up=up, s1_ident=s1_ident)
    host = dict(m=m, v1=v1, u=u, u2e=u2e, d2e=d2e, pad01=pad01, sv01=sv01,
                g=g_mat, gm=gm_mat)
    return meta, host


def _lay16(meta):
    """Column layout of the bf16 packed-constants tensor."""
    up, s2p = meta["up"], meta["v1n"] * meta["gmax"]
    pieces = [
        ("xut", P, KC * up),        # x[U]^T chunked  [128, KC*up]
        ("wsd1", P, KC * 2 * H1),   # [Ws1|Wd1] chunked
        ("u2e", up, s2p),
        ("d2e", up, s2p),
        ("pad01", 1, s2p),
        ("neg8", 1, H1),
        ("sv01", s2p, meta["v1n"] * up),
    ]
    lay, off = {}, 0
    for name, rows, cols in pieces:
        lay[name] = (rows, off, cols)
        off += cols
    return lay, off


def _lay32(meta):
    """Column layout of the f32 packed-constants tensor (tail/oxm)."""
    v1n, s1n = meta["v1n"], meta["s1n"]
    pieces = [
        ("xm", P, KC),
        ("wfb", P, KC * 2),
        ("g", v1n, s1n),
        ("gm", v1n, s1n),
        ("shiftrow", 1, s1n),
        ("one11", 1, 1),
        ("bias3s", 1, 2),
        ("ones_s1", s1n, 1),
    ]
    lay, off = {}, 0
    for name, rows, cols in pieces:
        lay[name] = (rows, off, cols)
        off += cols
    return lay, off


def _chunked(w):
    """[K, N] -> [128, (K//128)*N] chunk-major free layout."""
    k, n = w.shape
    assert k % P == 0
    return np.ascontiguousarray(
        w.reshape(k // P, P, n).transpose(1, 0, 2).reshape(P, (k // P) * n))


# ---------------------------------------------------------------- bass build
def _build(meta):
    v1n, s1n, gmax = meta["v1n"], meta["s1n"], meta["gmax"]
    up, s1_ident = meta["up"], meta["s1_ident"]
    s2p = v1n * gmax
    nblk = H1 * KC                  # 48 (head, f-chunk) output blocks
    lay16, cw16 = _lay16(meta)
    lay32, cw32 = _lay32(meta)

    nc = bacc.Bacc("TRN2", target_bir_lowering=False, debug=False,
                   enable_asserts=True, num_devices=NCORES)

    d_cst16 = nc.dram_tensor("cst16", [P, cw16], bf16, kind="ExternalInput")
    d_xu = nc.dram_tensor("xu", [up, C], bf16, kind="ExternalInput")
    d_w1b = nc.dram_tensor("w1b", [1, H1 * OUT], fp8, kind="ExternalInput")
    d_w1 = [nc.dram_tensor(f"w1c{c}", [P, H1 * OUT], fp8, kind="ExternalInput")
            for c in range(KC)]
    d_w2f = nc.dram_tensor("w2f", [P, nblk * 4], bf16, kind="ExternalInput")
    d_cst32 = nc.dram_tensor("cst32", [P, cw32], f32, kind="ExternalInput")
    d_res = nc.dram_tensor("res", [1, 2], f32, kind="ExternalOutput")

    with tile.TileContext(nc) as tc:
        with (
            tc.tile_pool(name="const", bufs=1) as cpool,
            tc.tile_pool(name="sbuf", bufs=1) as sb,
            tc.tile_pool(name="big", bufs=1) as bigp,
            tc.tile_pool(name="ps", bufs=1, space="PSUM") as ps,
        ):
            # ---- input DMAs. w1c0 first: its transfer hides the HWDGE
            # generation of the small attention tensors; the W1 stream then
            # owns the DMA engines back-to-back.
            w1_sb = [bigp.tile([P, H1 * OUT], fp8, tag=f"w1_{c}",
                               name=f"w1_{c}") for c in range(KC)]
            nc.sync.dma_start(out=w1_sb[0][:], in_=d_w1[0][:])
            cst16 = cpool.tile([P, cw16], bf16, tag="cst16")
            nc.sync.dma_start(out=cst16[:], in_=d_cst16[:])
            xu_sb = cpool.tile([up, C], bf16, tag="xu")
            nc.sync.dma_start(out=xu_sb[:], in_=d_xu[:])
            w1b_sb = cpool.tile([1, H1 * OUT], fp8, tag="w1b")
            nc.sync.dma_start(out=w1b_sb[:], in_=d_w1b[:])
            for c in range(1, KC):
                nc.sync.dma_start(out=w1_sb[c][:], in_=d_w1[c][:])
            w2f_sb = cpool.tile([P, nblk * 4], bf16, tag="w2f")
            nc.sync.dma_start(out=w2f_sb[:], in_=d_w2f[:])
            cst32 = cpool.tile([P, cw32], f32, tag="cst32")
            nc.sync.dma_start(out=cst32[:], in_=d_cst32[:])

            def cv16(name):
                rows, off, cols = lay16[name]
                return cst16[0:rows, off:off + cols]

            def cv32(name):
                rows, off, cols = lay32[name]
                return cst32[0:rows, off:off + cols]

            xut_v = cv16("xut").rearrange("p (k n) -> p k n", k=KC)
            wsd1_v = cv16("wsd1").rearrange("p (k n) -> p k n", k=KC)
            u2e_v = cv16("u2e")
            d2e_v = cv16("d2e")
            pad01_v = cv16("pad01")
            neg8_v = cv16("neg8")
            sv01_v = cv16("sv01")

            ident = cpool.tile([H1, H1], f32, tag="ident")
            make_identity(nc, ident[:])
            ones2 = cpool.tile([1, v1n], fp8, tag="ones2")
            nc.vector.memset(ones2[:], 1.0)

            # ---- attention: a_src/a_dst at U, all heads ----
            attb = ps.tile([P, 512], f32, tag="attbank")
            asd_ps = attb[0:up, 0:2 * H1]
            lg_ps = attb[0:H1, 16:16 + s2p]
            at_ps = attb[0:s2p, 144:144 + H1]
            wuv_ps = [attb[0:up, 152 + 8 * v:160 + 8 * v]
                      for v in range(v1n)]
            for c in range(KC):
                nc.tensor.matmul(out=asd_ps, lhsT=xut_v[:, c, :],
                                 rhs=wsd1_v[:, c, :],
                                 start=(c == 0), stop=(c == KC - 1))
            asd_sb = sb.tile([up, 2 * H1], bf16, tag="asd_sb")
            nc.vector.tensor_copy(out=asd_sb[:], in_=asd_ps)

            # per-edge logits: a_s[src_e] + a_d[dst_e] + pad bias
            nc.tensor.matmul(out=lg_ps, lhsT=asd_sb[:, 0:H1], rhs=u2e_v,
                             start=True, stop=False)
            nc.tensor.matmul(out=lg_ps, lhsT=asd_sb[:, H1:2 * H1],
                             rhs=d2e_v, start=False, stop=False)
            nc.tensor.matmul(out=lg_ps, lhsT=neg8_v, rhs=pad01_v,
                             start=False, stop=True)

            # leaky-relu (one Act op), exp without max-shift (logits tiny),
            # then per-group normalize
            lg_t = sb.tile([H1, s2p], f32, tag="lg_t")
            nc.vector.tensor_scalar_mul(out=lg_t[:], in0=lg_ps, scalar1=0.2)
            lg_sb = sb.tile([H1, s2p], f32, tag="lg_sb")
            nc.vector.tensor_tensor(out=lg_sb[:], in0=lg_ps, in1=lg_t[:],
                                    op=mybir.AluOpType.max)
            ee_sb = sb.tile([H1, s2p], f32, tag="ee_sb")
            nc.scalar.activation(out=ee_sb[:], in_=lg_sb[:],
                                 func=mybir.ActivationFunctionType.Exp)
            eev = ee_sb[:].rearrange("h (g e) -> h g e", e=gmax)
            den = sb.tile([H1, v1n], f32, tag="den")
            nc.vector.reduce_sum(out=den[:], in_=eev,
                                 axis=mybir.AxisListType.X)
            rec = sb.tile([H1, v1n], f32, tag="rec")
            nc.vector.reciprocal(out=rec[:], in_=den[:])
            alpha_sb = sb.tile([H1, s2p], f32, tag="alpha_sb")
            recb = rec[:].rearrange("h (g o) -> h g o", o=1).to_broadcast(
                [H1, v1n, gmax])
            nc.vector.tensor_tensor(
                out=alpha_sb[:].rearrange("h (g e) -> h g e", e=gmax),
                in0=eev, in1=recb, op=mybir.AluOpType.mult)

            # alpha^T via PE transpose, then wuv[(v,u), h] = sum_e alpha
            nc.tensor.transpose(out=at_ps, in_=alpha_sb[:],
                                identity=ident[:])
            at_sb = sb.tile([s2p, H1], bf16, tag="at_sb")
            nc.vector.tensor_copy(out=at_sb[:], in_=at_ps)
            # per-v blocks: PE/DVE partition bases must be 0/32/64-aligned
            wuv_sb = [sb.tile([up, H1], bf16, tag=f"wuv_sb{v}",
                              name=f"wuv_sb{v}") for v in range(v1n)]
            for v in range(v1n):
                nc.tensor.matmul(out=wuv_ps[v],
                                 lhsT=sv01_v[:, v * up:(v + 1) * up],
                                 rhs=at_sb[:], start=True, stop=True)
                nc.vector.tensor_copy(out=wuv_sb[v][:], in_=wuv_ps[v])

            # xagg^T chunks: [128c, (c,v,h)] = sum_u x[U]^T wuv
            xagg_ps = ps.tile([P, KC * v1n * H1], f32, tag="xagg")
            for c in range(KC):
                for v in range(v1n):
                    nc.tensor.matmul(
                        out=xagg_ps[:, (c * v1n + v) * H1:
                                    (c * v1n + v + 1) * H1],
                        lhsT=xu_sb[:, c * P:(c + 1) * P],
                        rhs=wuv_sb[v][:],
                        start=True, stop=True)
            xagg8 = sb.tile([P, KC * v1n * H1], fp8, tag="xagg8")
            nc.vector.tensor_copy(out=xagg8[:], in_=xagg_ps[:])
            xagg8_v = xagg8[:].rearrange("p (c v h) -> p c v h", c=KC, v=v1n)

            # ---- the big GEMM: agg[f, (h,fc,v)] = xagg @ (64*W1) + 64*b1
            # bias row first (start), then one accumulate pass per W1 chunk
            # as its DMA lands; fp8 x fp8 -> f32 PSUM.
            agg_ps = ps.tile([P, nblk * v1n], f32, tag="agg")
            for k in range(nblk):
                h, fc = divmod(k, KC)
                nc.tensor.matmul(
                    out=agg_ps[:, k * v1n:(k + 1) * v1n],
                    lhsT=w1b_sb[0:1, h * OUT + fc * P:h * OUT + (fc + 1) * P],
                    rhs=ones2[:], start=True, stop=False,
                    skip_group_check=True)
            for c in range(KC):
                for k in range(nblk):
                    h, fc = divmod(k, KC)
                    nc.tensor.matmul(
                        out=agg_ps[:, k * v1n:(k + 1) * v1n],
                        lhsT=w1_sb[c][:, h * OUT + fc * P:
                                      h * OUT + (fc + 1) * P],
                        rhs=xagg8_v[:, c, :, h],
                        start=False, stop=(c == KC - 1),
                        skip_group_check=True)

            # elu'(x) = elu(x) + 1 = max(x,0) + min(exp(x),1); x = agg/64.
            # The -1 is folded into host constants downstream.
            t1_sb = sb.tile([P, nblk * v1n], f32, tag="t1_sb")
            nc.vector.tensor_scalar(out=t1_sb[:], in0=agg_ps[:],
                                    scalar1=1.0 / W1SCALE, scalar2=0.0,
                                    op0=mybir.AluOpType.mult,
                                    op1=mybir.AluOpType.max)
            ee2_sb = sb.tile([P, nblk * v1n], f32, tag="ee2_sb")
            nc.scalar.activation(out=ee2_sb[:], in_=agg_ps[:],
                                 func=mybir.ActivationFunctionType.Exp,
                                 scale=1.0 / W1SCALE)
            nc.vector.tensor_scalar(out=ee2_sb[:], in0=ee2_sb[:],
                                    scalar1=1.0, scalar2=None,
                                    op0=mybir.AluOpType.min)
            helu_sb = sb.tile([P, nblk * v1n], bf16, tag="helu_sb")
            nc.vector.tensor_tensor(out=helu_sb[:], in0=t1_sb[:],
                                    in1=ee2_sb[:], op=mybir.AluOpType.add)

            # ---- oxm = x[m] @ wf_bot + bias3s (off critical path) ----
            xm_v = cv32("xm")
            wfb_v = cv32("wfb").rearrange("p (k n) -> p k n", k=KC)
            one11_v = cv32("one11")
            bias3s_v = cv32("bias3s")
            tailb = ps.tile([P, 12], f32, tag="tailbank")
            oxm_ps = tailb[0:1, 0:2]
            h2f_ps = tailb[0:v1n, 2:6]
            r2t_ps = tailb[0:s1n, 6:7]
            den_ps = tailb[0:1, 7:8]
            fin_ps = tailb[0:1, 8:10]
            for c in range(KC):
                nc.tensor.matmul(out=oxm_ps, lhsT=xm_v[:, c:c + 1],
                                 rhs=wfb_v[:, c, :],
                                 start=(c == 0), stop=False)
            nc.tensor.matmul(out=oxm_ps, lhsT=one11_v, rhs=bias3s_v,
                             start=False, stop=True)

            # ---- folded layer-2: h2f'[v, 0:4] = helu' @ [w2fold|Ws2|Wd2]
            for k in range(nblk):
                nc.tensor.matmul(out=h2f_ps,
                                 lhsT=helu_sb[:, k * v1n:(k + 1) * v1n],
                                 rhs=w2f_sb[:, k * 4:(k + 1) * 4],
                                 start=(k == 0), stop=(k == nblk - 1))
            h2f_sb = sb.tile([v1n, 4], f32, tag="h2f_sb")
            nc.vector.tensor_copy(out=h2f_sb[:], in_=h2f_ps)

            # ---- layer-2 logits (transposed), softmax, weighted sum ----
            g_v = cv32("g")
            gm_v = cv32("gm")
            shiftrow_v = cv32("shiftrow")
            ones_s1_v = cv32("ones_s1")
            nc.tensor.matmul(out=r2t_ps, lhsT=g_v, rhs=h2f_sb[:, 2:3],
                             start=True, stop=False)
            nc.tensor.matmul(out=r2t_ps, lhsT=gm_v, rhs=h2f_sb[:, 3:4],
                             start=False, stop=False)
            nc.tensor.matmul(out=r2t_ps, lhsT=shiftrow_v, rhs=one11_v,
                             start=False, stop=True)
            al2t_t = sb.tile([s1n, 1], f32, tag="al2t_t")
            nc.vector.tensor_scalar_mul(out=al2t_t[:], in0=r2t_ps,
                                        scalar1=0.2)
            al2t_sb = sb.tile([s1n, 1], f32, tag="al2t_sb")
            nc.vector.tensor_tensor(out=al2t_sb[:], in0=r2t_ps,
                                    in1=al2t_t[:], op=mybir.AluOpType.max)
            e2t_sb = sb.tile([s1n, 1], f32, tag="e2t_sb")
            nc.scalar.activation(out=e2t_sb[:], in_=al2t_sb[:],
                                 func=mybir.ActivationFunctionType.Exp)

            nc.tensor.matmul(out=den_ps, lhsT=e2t_sb[:], rhs=ones_s1_v,
                             start=True, stop=True)
            if s1_ident:
                nc.tensor.matmul(out=fin_ps, lhsT=e2t_sb[:],
                                 rhs=h2f_sb[:, 0:2], start=True, stop=True)
            else:
                gath_ps = tailb[0:s1n, 10:12]
                nc.tensor.matmul(out=gath_ps, lhsT=g_v,
                                 rhs=h2f_sb[:, 0:2], start=True, stop=True)
                gath_sb = sb.tile([s1n, 2], f32, tag="gath_sb")
                nc.vector.tensor_copy(out=gath_sb[:], in_=gath_ps)
                nc.tensor.matmul(out=fin_ps, lhsT=e2t_sb[:],
                                 rhs=gath_sb[:], start=True, stop=True)

            rec2 = sb.tile([1, 1], f32, tag="rec2")
            nc.vector.reciprocal(out=rec2[:], in_=den_ps)
            res1 = sb.tile([1, 2], f32, tag="res1")
            nc.vector.tensor_scalar(out=res1[:], in0=fin_ps,
                                    scalar1=rec2[:, 0:1], scalar2=None,
                                    op0=mybir.AluOpType.mult)
            res_sb = sb.tile([1, 2], f32, tag="res_sb")
            nc.vector.tensor_add(out=res_sb[:], in0=res1[:],
                                 in1=oxm_ps)
            nc.sync.dma_start(out=d_res[:], in_=res_sb[:])

    nc.compile()
    return nc


_CACHE = {}


def _get_nc(meta):
    key = repr(sorted(meta.items()))
    if key not in _CACHE:
        _CACHE[key] = _build(meta)
    return _CACHE[key]


def make_in_maps(**inputs):
    x = np.asarray(inputs["x"], np.float32)
    n_nodes = x.shape[0]
    meta, host = _preprocess(inputs["edge_index"], inputs["mask_idx"], n_nodes)
    v1n, s1n, up = meta["v1n"], meta["s1n"], meta["up"]
    s2p = v1n * meta["gmax"]
    nblk = H1 * KC

    W1 = np.asarray(inputs["W1"], np.float32)
    att_s1 = np.asarray(inputs["att_src1"], np.float32)
    att_d1 = np.asarray(inputs["att_dst1"], np.float32)
    b1 = np.asarray(inputs["b1"], np.float32)
    W2 = np.asarray(inputs["W2"], np.float32)
    att_s2 = np.asarray(inputs["att_src2"], np.float32)
    att_d2 = np.asarray(inputs["att_dst2"], np.float32)
    b2 = np.asarray(inputs["b2"], np.float32)
    fc_w = np.asarray(inputs["fc_w"], np.float32)
    fc_b = np.asarray(inputs["fc_b"], np.float32)
    cls_w = np.asarray(inputs["cls_w"], np.float32)
    cls_b = np.asarray(inputs["cls_b"], np.float32)

    # weight-weight folds
    Ws1 = np.einsum("chf,hf->ch", W1.reshape(C, H1, OUT), att_s1)   # [C, H1]
    Wd1 = np.einsum("chf,hf->ch", W1.reshape(C, H1, OUT), att_d1)
    Ws2 = W2 @ att_s2[0]                                            # [H1*OUT]
    Wd2 = W2 @ att_d2[0]
    wf = fc_w @ cls_w                                               # [1536, 2]
    wf_top, wf_bot = wf[:OUT], wf[OUT:]
    w2fold = W2 @ wf_top                                            # [6144, 2]
    # helu' = elu + 1 fold: subtract column sums; softmax shift constant
    shift_const = -(Ws2.sum() + Wd2.sum())
    bias3s = (b2 @ wf_top + fc_b @ cls_w + cls_b
              - w2fold.sum(axis=0)).reshape(1, 2).astype(np.float32)

    # w2f blocks ordered to match agg blocks k = h*KC + fc
    w2f4 = np.concatenate([w2fold, Ws2[:, None], Wd2[:, None]], axis=1)
    w2f_host = np.zeros((P, nblk * 4), np.float32)
    for k in range(nblk):
        w2f_host[:, k * 4:(k + 1) * 4] = w2f4[k * P:(k + 1) * P, :]

    # bf16 constants tensor
    lay16, cw16 = _lay16(meta)
    cst16 = np.zeros((P, cw16), np.float32)

    def fill16(name, arr):
        rows, off, cols = lay16[name]
        assert arr.shape == (rows, cols), (name, arr.shape, (rows, cols))
        cst16[0:rows, off:off + cols] = arr

    xu_rows = np.zeros((up, C), np.float32)
    xu_rows[:meta["un"]] = x[host["u"]]
    xut = np.zeros((P, KC * up), np.float32)
    for c in range(KC):
        xut[:, c * up:(c + 1) * up] = xu_rows[:, c * P:(c + 1) * P].T
    fill16("xut", xut)
    fill16("wsd1", _chunked(np.concatenate([Ws1, Wd1], axis=1)))
    fill16("u2e", host["u2e"])
    fill16("d2e", host["d2e"])
    fill16("pad01", host["pad01"])
    fill16("neg8", np.full((1, H1), NEGPAD, np.float32))
    fill16("sv01", host["sv01"])

    # f32 constants tensor (tail)
    lay32, cw32 = _lay32(meta)
    cst32 = np.zeros((P, cw32), np.float32)

    def fill32(name, arr):
        rows, off, cols = lay32[name]
        assert arr.shape == (rows, cols), (name, arr.shape, (rows, cols))
        cst32[0:rows, off:off + cols] = arr

    fill32("xm", np.ascontiguousarray(x[host["m"]].reshape(KC, P).T))
    fill32("wfb", _chunked(np.ascontiguousarray(wf_bot)))
    fill32("g", host["g"])
    fill32("gm", host["gm"])
    fill32("shiftrow", np.full((1, s1n), shift_const, np.float32))
    fill32("one11", np.ones((1, 1), np.float32))
    fill32("bias3s", bias3s)
    fill32("ones_s1", np.ones((s1n, 1), np.float32))

    w1s = (W1 * W1SCALE).astype(np_fp8)                 # [768, 6144] fp8
    w1b = (b1 * W1SCALE).astype(np_fp8).reshape(1, H1 * OUT)

    im = {
        "cst16": cst16.astype(np_bf16),
        "xu": xu_rows.astype(np_bf16),
        "w1b": w1b,
        "w2f": w2f_host.astype(np_bf16),
        "cst32": cst32,
    }
    for c in range(KC):
        im[f"w1c{c}"] = np.ascontiguousarray(w1s[c * P:(c + 1) * P, :])
    return meta, [im] * NCORES


def kernel(**inputs):
    meta, in_maps = make_in_maps(**inputs)
    nc = _get_nc(meta)
    res = run_bass_kernel_spmd(nc, in_maps, core_ids=list(range(NCORES)))
    return res.results[0]["res"].astype(np.float32)
